# revision 33
# baseline (speedup 1.0000x reference)
"""Trainium2 Bass kernel: batched int8 GEMM (bmm_s8t_s8n) with fused bf16 dequant.

Computes out[i] = bf16(alpha * (a[i] @ b[i]^T)) for a [32,512,2048] int8,
b [32,512,2048] int8 (both row-major with K innermost), alpha scalar fp32.

Strategy (per 8-core SPMD shard = 4 batches/core):
  1. Both operands need K on partitions for the PE.  DMA xbar transpose only
     supports 2-byte elements, so int8 pairs along K are viewed as uint16 and
     transposed chunk-by-chunk ([512 rows, 128 pair-cols] -> [128, 512]u16)
     into SBUF staging tiles.  A partition then holds two int8 k-slices
     byte-interleaved along the free dim; whatever (partition, col) mapping the
     xbar uses is identical for a and b, so the contraction is correct
     regardless.
  2. The xbar stream (~51us for 8.4MB at ~160GB/s) is the feed floor and the
     PE (~55us of matmuls at the 216ns N=512 issue floor) the compute floor;
     they overlap almost entirely.  Descriptor generation costs ~0.9us/call
     fixed + ~0.33us/chunk and must come from a single engine (concurrent
     transpose issue from two HWDGE engines races the xbar's shared base
     register and corrupts data - observed).  All gens go on SYNC: 2-chunk
     calls (~0.8us/chunk gen = xbar rate) keep the stream gapless; batch 0's
     first two chunks go per-chunk so the first matmul issues at ~11us.
  3. int8 -> bf16 de-interleave casts run at matching granularity on DVE
     (~0.7us/chunk) so k-tiles trail the stream by under a microsecond.
  4. PE prewarm: dummy matmuls on a zeroed scratch tile run during the fixed
     ~7us NEFF preamble so the HAM clock gate reaches 2.4GHz around when the
     first real matmul issues (~11us).
  5. Engine factorization so epilogues never head-of-line-block the feed:
     SYNC=transpose gens, DVE=casts, ACT=dequants, GPSIMD/SYNC=stores.  The
     tile scheduler statically serializes ALL DMA instructions across engines
     and enforces that order with completion semaphores, so non-tail stores
     are emitted after the whole batch loop ([all gens][all stores] order;
     outputs buffer in SBUF) - otherwise a late store stalls the transpose
     stream and convoys the whole pipeline (observed, +10us).
  6. t-major accumulation across 4 open PSUM banks per batch, per-m dequant
     immediately after each m-group's last k-tile; the final batch runs its
     last k-tile round in reversed m-order and the very last m-group's
     epilogue is quartered across ACT/DVE dequants and GPSIMD/SYNC stores to
     shorten the tail.

Measured (8 cores, NTFF): ~76.5us unthrottled (~89us when the chip is in its
2.0GHz P0 power-throttle regime), bitwise-exact output. The prior baseline
measured 81.1us / 93.7us in the same two regimes.
"""

from dataclasses import dataclass

import numpy as np

import concourse.mybir as mybir
from concourse import bacc
from concourse.bass_utils import run_bass_kernel_spmd
from concourse.tile import TileContext

B, M, N, K = 32, 512, 512, 2048
NCORES = 8
BPC = B // NCORES  # batches per core
KP = K // 2  # uint16 pair-columns per row
PART = 128
NCHUNK = KP // PART  # transposed chunks per operand-batch (8)
KTILES = 2 * NCHUNK  # k-tiles of 128 per batch (16)


SCHED0 = (1, 2, 2, 3)  # batch-0 chunk-group sizes (gens and casts)
SCHED = (4, 4)  # later batches


@dataclass(frozen=True)
class Cfg:
    stage_bufs: int = 4  # per tag [128, 4096]u16 staging tiles (8 KiB/partition)
    conv_bufs: int = 14  # per tag bf16 chunk tiles (up to 4 KiB/partition)
    prewarm: int = 10  # dummy N=512 matmuls to pull HAM to 2.4GHz early
    obuf_bufs: int = 16
    psum_bufs: int = 8
    split_tail: bool = True  # split final m-group epilogue in half across DVE+ACT
    b_casts: str = "vector"  # engine for b-operand casts ("vector"|"scalar"|"alt")
    memset_eng: str = "gpsimd"  # engine for the prewarm scratch memset
    sched0: tuple = SCHED0
    sched: tuple = (1, 3, 4)


VARIANTS = {
    "w1": Cfg(),
    "w2": Cfg(prewarm=0),
    "w3": Cfg(b_casts="alt"),
    "w4": Cfg(sched0=(4, 4), sched=(4, 4), conv_bufs=4),
    "w5": Cfg(prewarm=12),
    "w6": Cfg(split_tail=False),
    "w7": Cfg(sched0=(1, 1, 1, 1, 2, 2)),
    "w8": Cfg(sched0=(1, 1, 2, 4)),
    "w9": Cfg(sched=(2, 2, 2, 2), conv_bufs=8),
    "t8": Cfg(prewarm=8),
    "t12": Cfg(prewarm=12),
    "m14": Cfg(sched=(1, 1, 2, 4)),
    "m22": Cfg(sched=(2, 2, 4)),
    "s134": Cfg(sched=(1, 3, 4)),
    "gm": Cfg(memset_eng="gpsimd"),
    "s0a": Cfg(sched0=(1, 1, 2, 4)),
    "s0b": Cfg(sched0=(1, 2, 2, 3)),
    "c14": Cfg(conv_bufs=14),
    "c12": Cfg(conv_bufs=12),
}

_cfg = VARIANTS["w1"]


def set_variant(name):
    global _cfg
    _cfg = VARIANTS[name] if isinstance(name, str) else name


def _build(alpha: float, bpc: int = BPC):
    cfg = _cfg
    nc = bacc.Bacc("TRN2", target_bir_lowering=False)
    a_d = nc.dram_tensor("a", [bpc, M, KP], mybir.dt.uint16, kind="ExternalInput")
    b_d = nc.dram_tensor("b", [bpc, N, KP], mybir.dt.uint16, kind="ExternalInput")
    o_d = nc.dram_tensor("out", [bpc, M, N], mybir.dt.bfloat16, kind="ExternalOutput")

    n_mt = M // PART
    with TileContext(nc) as tc:
        with (
            tc.tile_pool(name="warm", bufs=1) as warm,
            tc.tile_pool(name="stage", bufs=cfg.stage_bufs) as stage,
            tc.tile_pool(name="conv", bufs=cfg.conv_bufs) as conv,
            tc.tile_pool(name="obuf", bufs=cfg.obuf_bufs) as obuf,
            tc.tile_pool(name="psum", bufs=cfg.psum_bufs, space="PSUM") as psum_pool,
        ):
            # --- PE prewarm: no input deps, runs during the NEFF preamble ---
            if cfg.prewarm:
                wt = warm.tile([PART, N], mybir.dt.bfloat16)
                getattr(nc, cfg.memset_eng).memset(wt[:, :], 0.0)
                wps = psum_pool.tile([PART, N], mybir.dt.float32, tag="ps")
                for _ in range(cfg.prewarm):
                    nc.tensor.matmul(
                        wps[:, :], wt[:, :PART], wt[:, :], start=True, stop=True
                    )

            def deq(ot, ps, deq_eng, lo=0, hi=N):
                # epilogue engines are ACT (+DVE for the very last half) so the
                # feed path (SYNC gens, DVE casts) is never head-of-line blocked
                if deq_eng is nc.scalar:
                    nc.scalar.activation(
                        ot[:, lo:hi],
                        ps[:, lo:hi],
                        mybir.ActivationFunctionType.Copy,
                        scale=float(alpha),
                    )
                else:
                    deq_eng.tensor_scalar_mul(ot[:, lo:hi], ps[:, lo:hi], float(alpha))

            deferred_stores = []  # (obuf_tile, bi, mi): emitted after all gens

            def epilogue(ps, bi, mi):
                ot = obuf.tile([PART, N], mybir.dt.bfloat16)
                deq(ot, ps, nc.scalar)
                deferred_stores.append((ot, bi, mi))

            for bi in range(bpc):
                sched = cfg.sched0 if bi == 0 else cfg.sched
                assert sum(sched) == NCHUNK
                sts = {}
                stt = {}
                for name in ("a", "b"):
                    st = stage.tile([PART, NCHUNK * M], mybir.dt.uint16, tag=f"st_{name}")
                    stt[name] = st
                    sts[name] = st[:, :].bitcast(mybir.dt.int8)  # [128, 2*KP]
                # transposes: all on SYNC (single issuer - the xbar path has
                # shared state), a/b interleaved per chunk-group
                c0 = 0
                for g in sched:
                    for name, dram in (("a", a_d), ("b", b_d)):
                        nc.sync.dma_start_transpose(
                            stt[name][:, c0 * M : (c0 + g) * M].rearrange(
                                "q (c m) -> q c m", m=M
                            ),
                            dram[bi, :, c0 * PART : (c0 + g) * PART],
                        )
                    c0 += g
                # casts: always per-chunk on DVE (finer k-tile availability
                # than the gen granularity; a chunk cast waits its gen's sem)
                ktiles = {"a": [], "b": []}
                c0 = 0
                for g in sched:
                    for name in ("a", "b"):
                        for c in range(c0, c0 + g):
                            chunk8 = sts[name][:, c * 2 * M : (c + 1) * 2 * M]
                            eng = nc.vector
                            bt = conv.tile(
                                [PART, 2 * M], mybir.dt.bfloat16, tag=f"bf_{name}"
                            )
                            # in: [q][c][m][p] bytes -> (c, p, m); out [c][p][m]
                            in_ap = chunk8.rearrange("q (c m p) -> q c p m", p=2, m=M)
                            out_ap = bt[:, :].rearrange(
                                "q (c p m) -> q c p m", m=M, p=2
                            )
                            if bi == 0 and c == 0:
                                # split the very first chunk's cast by parity so
                                # k-tile 0 is available ~0.35us sooner
                                for p in range(2):
                                    eng.tensor_copy(
                                        out=out_ap[:, :, p : p + 1, :],
                                        in_=in_ap[:, :, p : p + 1, :],
                                    )
                            else:
                                eng.tensor_copy(out=out_ap, in_=in_ap)
                            ktiles[name].append(bt)
                    c0 += g

                def mm(ps, mi, c, p, t):
                    nc.tensor.matmul(
                        ps[:, :],
                        ktiles["a"][c][:, p * M + mi * PART : p * M + (mi + 1) * PART],
                        ktiles["b"][c][:, p * N : (p + 1) * N],
                        start=(t == 0),
                        stop=(t == KTILES - 1),
                    )

                pss = [
                    psum_pool.tile(
                        [PART, N], mybir.dt.float32, name=f"ps_{bi}_{mi}", tag="ps"
                    )
                    for mi in range(n_mt)
                ]
                for t in range(KTILES - 1):
                    for mi in range(n_mt):
                        mm(pss[mi], mi, t // 2, t % 2, t)
                t = KTILES - 1
                # last batch: reversed m-order so the split epilogue lands on
                # the final m-group and earlier stores overlap remaining mms
                order = range(n_mt) if bi < bpc - 1 else range(n_mt - 1, -1, -1)
                for oi, mi in enumerate(order):
                    last = oi == n_mt - 1
                    mm(pss[mi], mi, t // 2, t % 2, t)
                    if bi < bpc - 1:
                        epilogue(pss[mi], bi, mi)
                    else:
                        # final batch: dequants alternate ACT/DVE and stores
                        # alternate GPSIMD/SYNC so the tail chains in parallel
                        ot = obuf.tile([PART, N], mybir.dt.bfloat16)
                        od = o_d[bi, mi * PART : (mi + 1) * PART, :]
                        s_eng = nc.gpsimd if oi % 2 == 0 else nc.sync
                        if not last or not cfg.split_tail:
                            deq(ot, pss[mi], nc.scalar if oi % 2 == 0 else nc.vector)
                            s_eng.dma_start(od, ot[:, :])
                        else:
                            # halve the very last epilogue across ACT then DVE
                            # (reads of one PSUM bank serialize, so more than
                            # two pieces only lengthens the chain); separate
                            # obuf tiles avoid a false WAR dep
                            ot2 = obuf.tile([PART, N], mybir.dt.bfloat16, tag="t2")
                            deq(ot, pss[mi], nc.scalar, 0, N // 2)
                            nc.gpsimd.dma_start(od[:, : N // 2], ot[:, : N // 2])
                            deq(ot2, pss[mi], nc.vector, N // 2, N)
                            nc.sync.dma_start(od[:, N // 2 :], ot2[:, N // 2 :])

            # non-tail stores issue from SYNC after every transpose gen, so the
            # scheduler's serialized static DMA order is [all gens][stores] and
            # late stores can never stall the transpose stream
            for ot, sbi, smi in deferred_stores:
                nc.sync.dma_start(o_d[sbi, smi * PART : (smi + 1) * PART, :], ot[:, :])
    nc.compile()
    return nc


def run(a, b, alpha, trace=False, repeats=1):
    """Run on 8 NeuronCores; returns (out [32,512,512] bf16, list[BassKernelResults])."""
    a = np.ascontiguousarray(np.asarray(a))
    b = np.ascontiguousarray(np.asarray(b))
    if a.dtype != np.int8:
        a = a.astype(np.int8)
    if b.dtype != np.int8:
        b = b.astype(np.int8)
    nc = _build(float(alpha))
    in_maps = []
    for ci in range(NCORES):
        sl = slice(ci * BPC, (ci + 1) * BPC)
        in_maps.append({"a": a[sl].view(np.uint16), "b": b[sl].view(np.uint16)})
    all_res = []
    for _ in range(repeats):
        res = run_bass_kernel_spmd(
            nc, in_maps, core_ids=list(range(NCORES)), trace=trace
        )
        all_res.append(res)
    out = np.concatenate([r["out"] for r in all_res[-1].results], axis=0)
    return out, all_res


def kernel(a, b, alpha):
    out, _ = run(a, b, alpha)
    return out
